# revision 11
# baseline (speedup 1.0000x reference)
"""Embedding-lookup kernel for Trainium2 (8 NeuronCores, SPMD batch-parallel).

Problem (hardcoded): B=4096, L=50, V=100000, D=64.
  - 4 "hist" tables [V, D]: gather [B, L, D], mean over L -> [B, D]
  - 4 "cat" tables  [V, D]: gather [B, 1, D]            -> [B, D]
  - output: concat -> [B, 8*D] = [4096, 512] float32

Architecture (per core, 512 batch rows):
  * All 8 tables stacked host-side into one [800000, 64] f32 DRAM tensor.
  * 104,448 lookups/core, each tagged with a dest id d in [0, 4096):
      hist: d = t*512 + b_local; cat: d = 2048 + t*512 + b_local.
  * Value space split into 25 windows of 32,000 rows (int16-addressable with
    per-instruction base).  Lookups sorted by (window, dest); each window
    gathered with ONE dma_gather (SWDGE custom op, the only primitive whose
    descriptor generation is fast enough).
  * Gathered slots land partition-minor: slot j -> [j%128, j//128].  Each
    128-slot "column" is reduced into a PSUM accumulator [128, 32*64] (dest d
    at [d%128, (d//128)*64:...]) via a selection matmul: DVE is_equal of the
    (host-precomputed, group-biased) dest stream against a static iota row
    builds the 0/1 mask, PE matmul accumulates.  This is the tile_scatter_add
    idiom (dma_scatter_add loses colliding updates on HW, measured).
  * SPMD: all cores share one instruction stream.  Window sizes are padded to
    the cross-core max; per-column group lists are the union across cores
    (a core lacking a group just contributes an all-zero mask).
  * Epilogue: PSUM -> SBUF with 1/L scale on the hist half, reassemble to
    [128, 512] tiles, DMA out.
"""

import numpy as np

B, L, V, D = 4096, 50, 100000, 64
NCORES = 8
BPC = B // NCORES            # 512 batch rows per core
P = 128
NTAB = 8                     # 4 hist + 4 cat
VSTACK = NTAB * V            # 800000
WROWS = 32000                # window size (int16-addressable)
NW = VSTACK // WROWS         # 25
NDEST = 4096                 # per-core dest ids
NG = NDEST // P              # 32 psum groups
NLOOK = BPC * (4 * L + 4)    # 104448 lookups per core

_cache = {}


def _prep(inputs):
    """Host prep: build per-core gather/bias streams + the shared structure."""
    hist = [np.asarray(inputs[f"hist{i}"], dtype=np.int64) for i in range(4)]
    cat = [np.asarray(inputs[f"cat{i}"], dtype=np.int64) for i in range(4)]
    w_hist = [np.asarray(inputs[f"W_hist{i}"], dtype=np.float32) for i in range(4)]
    w_cat = [np.asarray(inputs[f"W_cat{i}"], dtype=np.float32) for i in range(4)]
    tf32 = np.concatenate(w_hist + w_cat, axis=0)
    import ml_dtypes
    hi = tf32.astype(ml_dtypes.bfloat16)
    lo = (tf32 - hi.astype(np.float32)).astype(ml_dtypes.bfloat16)
    table = np.ascontiguousarray(np.concatenate([hi, lo], axis=1))  # [VSTACK, 2D] bf16

    # per-core (value, dest) pairs
    vals = np.empty((NCORES, NLOOK), np.int64)
    dsts = np.empty((NCORES, NLOOK), np.int64)
    for c in range(NCORES):
        b0 = c * BPC
        vparts, dparts = [], []
        for t in range(4):
            v = (hist[t][b0 : b0 + BPC] + t * V).ravel()            # [BPC*L]
            d = np.repeat(np.arange(BPC), L) + t * BPC
            vparts.append(v)
            dparts.append(d)
        for t in range(4):
            v = (cat[t][b0 : b0 + BPC] + (4 + t) * V).ravel()       # [BPC]
            d = np.arange(BPC) + 2048 + t * BPC
            vparts.append(v)
            dparts.append(d)
        vals[c] = np.concatenate(vparts)
        dsts[c] = np.concatenate(dparts)

    win = vals // WROWS
    order = np.lexsort((dsts, win), axis=-1)  # sort by (window, dest)
    vals = np.take_along_axis(vals, order, axis=1)
    dsts = np.take_along_axis(dsts, order, axis=1)
    win = np.take_along_axis(win, order, axis=1)

    # cross-core window sizes (in columns of 128)
    counts = np.stack([np.bincount(win[c], minlength=NW) for c in range(NCORES)])
    s_w = [int(np.ceil(counts[:, w].max() / P)) for w in range(NW)]

    # per-core padded per-window streams: local idx (int16) and dest (int32)
    idx_stream = np.zeros((NCORES, sum(s_w) * P), np.int16)
    dst_stream = np.full((NCORES, sum(s_w) * P), -1, np.int32)
    offs = np.concatenate([[0], np.cumsum([s * P for s in s_w])])
    for c in range(NCORES):
        pos = np.concatenate([[0], np.cumsum(counts[c])])
        for w in range(NW):
            n = counts[c, w]
            sl = slice(offs[w], offs[w] + n)
            idx_stream[c, sl] = (vals[c, pos[w] : pos[w] + n] - w * WROWS).astype(
                np.int16
            )
            dst_stream[c, sl] = dsts[c, pos[w] : pos[w] + n]

    # per-column group unions (shared structure)
    ncols = sum(s_w)
    dcols = dst_stream.reshape(NCORES, ncols, P)  # [(core), col, p] NOTE: col-major!
    col_groups = []
    for cidx in range(ncols):
        g = dcols[:, cidx, :]
        g = g[g >= 0] // P
        col_groups.append(sorted(set(int(x) for x in np.unique(g))))

    # bias streams: one f32 column per (col, group-in-union)
    nbias = sum(max(len(g), 0) for g in col_groups)
    bias = np.empty((NCORES, nbias, P), np.float32)
    j = 0
    for cidx, groups in enumerate(col_groups):
        for g in groups:
            bias[:, j, :] = dcols[:, cidx, :] - P * g
            j += 1
    assert j == nbias

    # device layouts: slot (p, col) = stream position col*128 + p
    # -> SBUF tile [128, ncols]: arr.reshape(ncols, 128).T
    idx_dev = np.empty((NCORES, P, sum(s_w) * P // 16), np.int16)
    for c in range(NCORES):
        flat = idx_stream[c]
        # 16-partition wrap per window, replicated to 128 partitions
        parts = []
        for w in range(NW):
            seg = flat[offs[w] : offs[w + 1]]
            wrapped = seg.reshape(-1, 16).T  # [16, NI_w/16]
            parts.append(np.tile(wrapped, (8, 1)))
        idx_dev[c] = np.concatenate(parts, axis=1)
    import ml_dtypes as _mld
    bias_dev = np.ascontiguousarray(
        np.transpose(bias, (0, 2, 1))
    ).astype(_mld.bfloat16)  # [core, 128, nbias]

    iota = np.tile(np.arange(P, dtype=np.float32)[None, :], (P, 1)).astype(
        _mld.bfloat16
    )

    meta = {
        "s_w": s_w,
        "col_groups": col_groups,
        "offs": offs,
    }
    in_maps = [
        {
            "table": table,
            "gidx": np.ascontiguousarray(idx_dev[c]),
            "bias": bias_dev[c],
            "iota": iota,
        }
        for c in range(NCORES)
    ]
    return meta, in_maps


def _build(meta):
    from concourse import bacc, mybir
    from concourse.tile import TileContext

    s_w = meta["s_w"]
    col_groups = meta["col_groups"]
    ncols = sum(s_w)
    nbias = sum(len(g) for g in col_groups)

    nc = bacc.Bacc(
        "TRN2",
        target_bir_lowering=False,
        debug=False,
        num_devices=NCORES,
        num_swdge_queues=4,
    )
    table = nc.dram_tensor(
        "table", [VSTACK, 2 * D], mybir.dt.bfloat16, kind="ExternalInput"
    ).ap()
    gidx = nc.dram_tensor(
        "gidx", [P, ncols * P // 16], mybir.dt.int16, kind="ExternalInput"
    ).ap()
    bias = nc.dram_tensor(
        "bias", [P, nbias], mybir.dt.bfloat16, kind="ExternalInput"
    ).ap()
    iota = nc.dram_tensor("iota", [P, P], mybir.dt.bfloat16, kind="ExternalInput").ap()
    out = nc.dram_tensor(
        "out", [BPC, NTAB * D], mybir.dt.float32, kind="ExternalOutput"
    ).ap()

    with TileContext(nc) as tc:
        with (
            tc.tile_pool(name="cst", bufs=1) as cst,
            tc.tile_pool(name="ip", bufs=4) as ip,
            tc.tile_pool(name="gp", bufs=6) as gp,
            tc.tile_pool(name="mp", bufs=12) as mp,
            tc.tile_pool(name="pp", bufs=1, space="PSUM") as pp,
            tc.tile_pool(name="op", bufs=2) as op,
        ):
            it = cst.tile([P, P], mybir.dt.bfloat16)
            nc.sync.dma_start(out=it[:], in_=iota[:])
            # preload bias columns once; indices loaded per half-window
            bias_all = cst.tile([P, nbias], mybir.dt.bfloat16)
            nc.sync.dma_start(out=bias_all[:], in_=bias[:])
            acc = pp.tile([P, NG * 2 * D], mybir.dt.float32, space="PSUM")
            nc.vector.memset(acc[:], 0.0)

            col0 = 0  # running column index
            bj = 0    # running bias column index
            ioff = 0  # running gidx column offset (int16 cols, = ncols*8 total)
            for w in range(NW):
                sw = s_w[w]
                ni = sw * P
                gt = gp.tile([P, sw, 2 * D], mybir.dt.bfloat16, tag="gt")
                sh = (sw + 1) // 2
                for h, (s0, s1) in enumerate(((0, sh), (sh, sw))):
                    if s1 <= s0:
                        continue
                    nh = (s1 - s0) * P
                    idx_t = ip.tile([P, nh // 16], mybir.dt.int16, tag="idx")
                    nc.sync.dma_start(
                        out=idx_t[:],
                        in_=gidx[:, ioff + s0 * 8 : ioff + s1 * 8],
                    )
                    nc.gpsimd.dma_gather(
                        gt[:, s0:s1, :],
                        table[w * WROWS : (w + 1) * WROWS, :],
                        idx_t[:],
                        nh,
                        nh,
                        2 * D,
                        single_packet=False,
                        queue_num=(2 * w + h) % 4,
                    )
                for s in range(sw):
                    for g in col_groups[col0 + s]:
                        sel = mp.tile([P, P], mybir.dt.bfloat16, tag="sel")
                        nc.vector.tensor_tensor(
                            out=sel[:],
                            in0=bias_all[:, bj : bj + 1].to_broadcast([P, P]),
                            in1=it[:],
                            op=mybir.AluOpType.is_equal,
                        )
                        nc.tensor.matmul(
                            out=acc[:, g * 2 * D : (g + 1) * 2 * D],
                            lhsT=sel[:],
                            rhs=gt[:, s, :],
                            start=False,
                            stop=True,
                        )
                        bj += 1
                col0 += sw
                ioff += ni // 16
            assert col0 == ncols and bj == nbias

            accv2 = acc[:].rearrange("p (g h e) -> p g h e", g=NG, h=2)
            for bb in range(BPC // P):
                # groups for this output tile: hist bb,4+bb,8+bb,12+bb; cat 16+...
                cb = op.tile([P, NTAB * D], mybir.dt.float32, tag="cb")
                cbv = cb[:].rearrange("p (t e) -> p t e", t=NTAB)
                nc.vector.tensor_copy(
                    out=cbv, in_=accv2[:, bb : NG : 4, 0, :]
                )
                nc.vector.tensor_add(
                    out=cbv, in0=cbv, in1=accv2[:, bb : NG : 4, 1, :]
                )
                ot = op.tile([P, NTAB * D], mybir.dt.float32, tag="ot")
                nc.scalar.mul(
                    out=ot[:, : 4 * D], in_=cb[:, : 4 * D], mul=1.0 / L
                )
                nc.vector.tensor_copy(out=ot[:, 4 * D :], in_=cb[:, 4 * D :])
                nc.sync.dma_start(out=out[bb * P : (bb + 1) * P, :], in_=ot[:])
    nc.compile()
    return nc


def _emulate(meta, in_maps):
    """Numpy emulation of the device program (for host-prep validation)."""
    s_w = meta["s_w"]
    col_groups = meta["col_groups"]
    outs = []
    for m in in_maps:
        table = m["table"]
        acc = np.zeros((P, NG * D), np.float32)
        ncols = sum(s_w)
        # reconstruct slot data from gidx (inverse of 16-wrap)
        col0 = 0
        ioff = 0
        bj = 0
        for w, sw in enumerate(s_w):
            ni = sw * P
            wrapped = m["gidx"][:16, ioff : ioff + ni // 16]
            flat = wrapped.T.reshape(-1)  # j = s*16 + p
            rows = table[w * WROWS + flat.astype(np.int64)].astype(np.float32)
            rows = rows[:, :D] + rows[:, D:]  # hi + lo
            for s in range(sw):
                colrows = rows[s * P : (s + 1) * P]  # [128, D], slot p
                for g in col_groups[col0 + s]:
                    bias_col = m["bias"][:, bj]  # [128]
                    selmask = np.zeros((P, P), np.float32)
                    for p in range(P):
                        v = bias_col[p]
                        if 0 <= v < P and v == int(v):
                            selmask[p, int(v)] = 1.0
                    acc[:, g * D : (g + 1) * D] += selmask.T @ colrows
                    bj += 1
            col0 += sw
            ioff += ni // 16
        o = np.empty((BPC, NTAB * D), np.float32)
        accv = acc.reshape(P, NG, D)
        for bb in range(BPC // P):
            o[bb * P : (bb + 1) * P, : 4 * D] = (
                accv[:, bb:16:4, :].reshape(P, 4 * D) / L
            )
            o[bb * P : (bb + 1) * P, 4 * D :] = accv[:, 16 + bb : 32 : 4, :].reshape(
                P, 4 * D
            )
        outs.append(o)
    return np.concatenate(outs, axis=0)


def _run(inputs, emulate=False, **spmd_kwargs):
    key = "nc"
    meta, in_maps = _prep(inputs)
    if emulate:
        return _emulate(meta, in_maps), None
    if key not in _cache:
        _cache[key] = _build(meta)
    from concourse.bass_utils import run_bass_kernel_spmd

    res = run_bass_kernel_spmd(
        _cache[key], in_maps, core_ids=list(range(NCORES)), **spmd_kwargs
    )
    outp = np.concatenate(
        [res.results[c]["out"] for c in range(NCORES)], axis=0
    )
    return outp, res


def kernel(**inputs) -> np.ndarray:
    outp, _ = _run(inputs)
    return outp


# revision 12
# speedup vs baseline: 1.7221x; 1.7221x over previous
"""Embedding-lookup kernel for Trainium2 (8 NeuronCores, SPMD batch-parallel).

Problem (hardcoded): B=4096, L=50, V=100000, D=64.
  - 4 "hist" tables [V, D]: gather [B, L, D], mean over L -> [B, D]
  - 4 "cat" tables  [V, D]: gather [B, 1, D]            -> [B, D]
  - output: concat -> [B, 8*D] = [4096, 512] float32

Architecture (per core, 512 batch rows):
  * All 8 tables stacked host-side into one [800000, 64] f32 DRAM tensor.
  * 104,448 lookups/core, each tagged with a dest id d in [0, 4096):
      hist: d = t*512 + b_local; cat: d = 2048 + t*512 + b_local.
  * Value space split into 25 windows of 32,000 rows (int16-addressable with
    per-instruction base).  Lookups sorted by (window, dest); each window
    gathered with ONE dma_gather (SWDGE custom op, the only primitive whose
    descriptor generation is fast enough).
  * Gathered slots land partition-minor: slot j -> [j%128, j//128].  Each
    128-slot "column" is reduced into a PSUM accumulator [128, 32*64] (dest d
    at [d%128, (d//128)*64:...]) via a selection matmul: DVE is_equal of the
    (host-precomputed, group-biased) dest stream against a static iota row
    builds the 0/1 mask, PE matmul accumulates.  This is the tile_scatter_add
    idiom (dma_scatter_add loses colliding updates on HW, measured).
  * SPMD: all cores share one instruction stream.  Window sizes are padded to
    the cross-core max; per-column group lists are the union across cores
    (a core lacking a group just contributes an all-zero mask).
  * Epilogue: PSUM -> SBUF with 1/L scale on the hist half, reassemble to
    [128, 512] tiles, DMA out.
"""

import numpy as np

B, L, V, D = 4096, 50, 100000, 64
NCORES = 8
BPC = B // NCORES            # 512 batch rows per core
P = 128
NTAB = 8                     # 4 hist + 4 cat
VSTACK = NTAB * V            # 800000
WROWS = 32000                # window size (int16-addressable)
NW = VSTACK // WROWS         # 25
NDEST = 4096                 # per-core dest ids
NG = NDEST // P              # 32 psum groups
NLOOK = BPC * (4 * L + 4)    # 104448 lookups per core

_cache = {}


def _prep(inputs):
    """Host prep: build per-core gather/bias streams + the shared structure."""
    hist = [np.asarray(inputs[f"hist{i}"], dtype=np.int64) for i in range(4)]
    cat = [np.asarray(inputs[f"cat{i}"], dtype=np.int64) for i in range(4)]
    w_hist = [np.asarray(inputs[f"W_hist{i}"], dtype=np.float32) for i in range(4)]
    w_cat = [np.asarray(inputs[f"W_cat{i}"], dtype=np.float32) for i in range(4)]
    tf32 = np.concatenate(w_hist + w_cat, axis=0)
    import ml_dtypes
    hi = tf32.astype(ml_dtypes.bfloat16)
    lo = (tf32 - hi.astype(np.float32)).astype(ml_dtypes.bfloat16)
    table = np.ascontiguousarray(np.concatenate([hi, lo], axis=1))  # [VSTACK, 2D] bf16

    # per-core (value, dest) pairs
    vals = np.empty((NCORES, NLOOK), np.int64)
    dsts = np.empty((NCORES, NLOOK), np.int64)
    for c in range(NCORES):
        b0 = c * BPC
        vparts, dparts = [], []
        for t in range(4):
            v = (hist[t][b0 : b0 + BPC] + t * V).ravel()            # [BPC*L]
            d = np.repeat(np.arange(BPC), L) + t * BPC
            vparts.append(v)
            dparts.append(d)
        for t in range(4):
            v = (cat[t][b0 : b0 + BPC] + (4 + t) * V).ravel()       # [BPC]
            d = np.arange(BPC) + 2048 + t * BPC
            vparts.append(v)
            dparts.append(d)
        vals[c] = np.concatenate(vparts)
        dsts[c] = np.concatenate(dparts)

    win = vals // WROWS
    order = np.lexsort((dsts, win), axis=-1)  # sort by (window, dest)
    vals = np.take_along_axis(vals, order, axis=1)
    dsts = np.take_along_axis(dsts, order, axis=1)
    win = np.take_along_axis(win, order, axis=1)

    # cross-core window sizes (in columns of 128)
    counts = np.stack([np.bincount(win[c], minlength=NW) for c in range(NCORES)])
    s_w = [int(np.ceil(counts[:, w].max() / P)) for w in range(NW)]

    # per-core padded per-window streams: local idx (int16) and dest (int32)
    idx_stream = np.zeros((NCORES, sum(s_w) * P), np.int16)
    dst_stream = np.full((NCORES, sum(s_w) * P), -1, np.int32)
    offs = np.concatenate([[0], np.cumsum([s * P for s in s_w])])
    for c in range(NCORES):
        pos = np.concatenate([[0], np.cumsum(counts[c])])
        for w in range(NW):
            n = counts[c, w]
            sl = slice(offs[w], offs[w] + n)
            idx_stream[c, sl] = (vals[c, pos[w] : pos[w] + n] - w * WROWS).astype(
                np.int16
            )
            dst_stream[c, sl] = dsts[c, pos[w] : pos[w] + n]

    # per-column group unions (shared structure)
    ncols = sum(s_w)
    dcols = dst_stream.reshape(NCORES, ncols, P)  # [(core), col, p] NOTE: col-major!
    col_groups = []
    for cidx in range(ncols):
        g = dcols[:, cidx, :]
        g = g[g >= 0] // P
        col_groups.append(sorted(set(int(x) for x in np.unique(g))))

    # bias streams: one f32 column per (col, group-in-union)
    nbias = sum(max(len(g), 0) for g in col_groups)
    bias = np.empty((NCORES, nbias, P), np.float32)
    j = 0
    for cidx, groups in enumerate(col_groups):
        for g in groups:
            bias[:, j, :] = dcols[:, cidx, :] - P * g
            j += 1
    assert j == nbias

    # device layouts: slot (p, col) = stream position col*128 + p
    # -> SBUF tile [128, ncols]: arr.reshape(ncols, 128).T
    idx_dev = np.empty((NCORES, P, sum(s_w) * P // 16), np.int16)
    for c in range(NCORES):
        flat = idx_stream[c]
        # 16-partition wrap per window, replicated to 128 partitions
        parts = []
        for w in range(NW):
            seg = flat[offs[w] : offs[w + 1]]
            wrapped = seg.reshape(-1, 16).T  # [16, NI_w/16]
            parts.append(np.tile(wrapped, (8, 1)))
        idx_dev[c] = np.concatenate(parts, axis=1)
    import ml_dtypes as _mld
    bias_dev = np.ascontiguousarray(
        np.transpose(bias, (0, 2, 1))
    ).astype(_mld.bfloat16)  # [core, 128, nbias]

    iota = np.tile(np.arange(P, dtype=np.float32)[None, :], (P, 1)).astype(
        _mld.bfloat16
    )

    meta = {
        "s_w": s_w,
        "col_groups": col_groups,
        "offs": offs,
    }
    in_maps = [
        {
            "table": table,
            "gidx": np.ascontiguousarray(idx_dev[c]),
            "bias": bias_dev[c],
            "iota": iota,
        }
        for c in range(NCORES)
    ]
    return meta, in_maps


def _build(meta):
    from concourse import bacc, mybir
    from concourse.tile import TileContext

    s_w = meta["s_w"]
    col_groups = meta["col_groups"]
    ncols = sum(s_w)
    nbias = sum(len(g) for g in col_groups)

    nc = bacc.Bacc(
        "TRN2",
        target_bir_lowering=False,
        debug=False,
        num_devices=NCORES,
        num_swdge_queues=4,
    )
    table = nc.dram_tensor(
        "table", [VSTACK, 2 * D], mybir.dt.bfloat16, kind="ExternalInput"
    ).ap()
    gidx = nc.dram_tensor(
        "gidx", [P, ncols * P // 16], mybir.dt.int16, kind="ExternalInput"
    ).ap()
    bias = nc.dram_tensor(
        "bias", [P, nbias], mybir.dt.bfloat16, kind="ExternalInput"
    ).ap()
    iota = nc.dram_tensor("iota", [P, P], mybir.dt.bfloat16, kind="ExternalInput").ap()
    out = nc.dram_tensor(
        "out", [BPC, NTAB * D], mybir.dt.float32, kind="ExternalOutput"
    ).ap()

    with TileContext(nc) as tc:
        with (
            tc.tile_pool(name="cst", bufs=1) as cst,
            tc.tile_pool(name="gp", bufs=6) as gp,
            tc.tile_pool(name="mp", bufs=12) as mp,
            tc.tile_pool(name="pp", bufs=1, space="PSUM") as pp,
            tc.tile_pool(name="op", bufs=2) as op,
        ):
            it = cst.tile([P, P], mybir.dt.bfloat16)
            nc.sync.dma_start(out=it[:], in_=iota[:])
            # preload ALL gather indices and bias columns once
            idx_all = cst.tile([P, ncols * P // 16], mybir.dt.int16)
            nc.sync.dma_start(out=idx_all[:], in_=gidx[:])
            bias_all = cst.tile([P, nbias], mybir.dt.bfloat16)
            nc.sync.dma_start(out=bias_all[:], in_=bias[:])
            acc = pp.tile([P, NG * 2 * D], mybir.dt.float32, space="PSUM")
            nc.vector.memset(acc[:], 0.0)

            col0 = 0  # running column index
            bj = 0    # running bias column index
            ioff = 0  # running gidx column offset (int16 cols, = ncols*8 total)
            for w in range(NW):
                sw = s_w[w]
                ni = sw * P
                gt = gp.tile([P, sw, 2 * D], mybir.dt.bfloat16, tag="gt")
                sh = (sw + 1) // 2
                for h, (s0, s1) in enumerate(((0, sh), (sh, sw))):
                    if s1 <= s0:
                        continue
                    nh = (s1 - s0) * P
                    nc.gpsimd.dma_gather(
                        gt[:, s0:s1, :],
                        table[w * WROWS : (w + 1) * WROWS, :],
                        idx_all[:, ioff + s0 * 8 : ioff + s1 * 8],
                        nh,
                        nh,
                        2 * D,
                        single_packet=False,
                        queue_num=(2 * w + h) % 4,
                    )
                for s in range(sw):
                    for g in col_groups[col0 + s]:
                        sel = mp.tile([P, P], mybir.dt.bfloat16, tag="sel")
                        nc.vector.tensor_tensor(
                            out=sel[:],
                            in0=bias_all[:, bj : bj + 1].to_broadcast([P, P]),
                            in1=it[:],
                            op=mybir.AluOpType.is_equal,
                        )
                        nc.tensor.matmul(
                            out=acc[:, g * 2 * D : (g + 1) * 2 * D],
                            lhsT=sel[:],
                            rhs=gt[:, s, :],
                            start=False,
                            stop=True,
                        )
                        bj += 1
                col0 += sw
                ioff += ni // 16
            assert col0 == ncols and bj == nbias

            accv2 = acc[:].rearrange("p (g h e) -> p g h e", g=NG, h=2)
            for bb in range(BPC // P):
                # groups for this output tile: hist bb,4+bb,8+bb,12+bb; cat 16+...
                cb = op.tile([P, NTAB * D], mybir.dt.float32, tag="cb")
                cbv = cb[:].rearrange("p (t e) -> p t e", t=NTAB)
                nc.vector.tensor_copy(
                    out=cbv, in_=accv2[:, bb : NG : 4, 0, :]
                )
                nc.vector.tensor_add(
                    out=cbv, in0=cbv, in1=accv2[:, bb : NG : 4, 1, :]
                )
                ot = op.tile([P, NTAB * D], mybir.dt.float32, tag="ot")
                nc.scalar.mul(
                    out=ot[:, : 4 * D], in_=cb[:, : 4 * D], mul=1.0 / L
                )
                nc.vector.tensor_copy(out=ot[:, 4 * D :], in_=cb[:, 4 * D :])
                nc.sync.dma_start(out=out[bb * P : (bb + 1) * P, :], in_=ot[:])
    nc.compile()
    return nc


def _emulate(meta, in_maps):
    """Numpy emulation of the device program (for host-prep validation)."""
    s_w = meta["s_w"]
    col_groups = meta["col_groups"]
    outs = []
    for m in in_maps:
        table = m["table"]
        acc = np.zeros((P, NG * D), np.float32)
        ncols = sum(s_w)
        # reconstruct slot data from gidx (inverse of 16-wrap)
        col0 = 0
        ioff = 0
        bj = 0
        for w, sw in enumerate(s_w):
            ni = sw * P
            wrapped = m["gidx"][:16, ioff : ioff + ni // 16]
            flat = wrapped.T.reshape(-1)  # j = s*16 + p
            rows = table[w * WROWS + flat.astype(np.int64)].astype(np.float32)
            rows = rows[:, :D] + rows[:, D:]  # hi + lo
            for s in range(sw):
                colrows = rows[s * P : (s + 1) * P]  # [128, D], slot p
                for g in col_groups[col0 + s]:
                    bias_col = m["bias"][:, bj]  # [128]
                    selmask = np.zeros((P, P), np.float32)
                    for p in range(P):
                        v = bias_col[p]
                        if 0 <= v < P and v == int(v):
                            selmask[p, int(v)] = 1.0
                    acc[:, g * D : (g + 1) * D] += selmask.T @ colrows
                    bj += 1
            col0 += sw
            ioff += ni // 16
        o = np.empty((BPC, NTAB * D), np.float32)
        accv = acc.reshape(P, NG, D)
        for bb in range(BPC // P):
            o[bb * P : (bb + 1) * P, : 4 * D] = (
                accv[:, bb:16:4, :].reshape(P, 4 * D) / L
            )
            o[bb * P : (bb + 1) * P, 4 * D :] = accv[:, 16 + bb : 32 : 4, :].reshape(
                P, 4 * D
            )
        outs.append(o)
    return np.concatenate(outs, axis=0)


def _run(inputs, emulate=False, **spmd_kwargs):
    key = "nc"
    meta, in_maps = _prep(inputs)
    if emulate:
        return _emulate(meta, in_maps), None
    if key not in _cache:
        _cache[key] = _build(meta)
    from concourse.bass_utils import run_bass_kernel_spmd

    res = run_bass_kernel_spmd(
        _cache[key], in_maps, core_ids=list(range(NCORES)), **spmd_kwargs
    )
    outp = np.concatenate(
        [res.results[c]["out"] for c in range(NCORES)], axis=0
    )
    return outp, res


def kernel(**inputs) -> np.ndarray:
    outp, _ = _run(inputs)
    return outp
